# revision 22
# baseline (speedup 1.0000x reference)
"""Trainium2 Bass kernel for nn_DKT (GAT chain-graph + LSTM network).

Strategy: data-parallel over batch (8 sequences per core x 8 cores).
Per core, tokens live on a padded grid t = s*512 + n (n < 499 real).
All activations are feature-major ([feature-partition, token-free]) so every
dense layer is a PE matmul with bf16 operands and fp32 PSUM accumulation.

Per core:
  - embedding lookups via custom dma_gather (transpose mode): gathers rows of
    bf16 tables, landing feature-major.
  - GAT layers: the graph is a bidirectional chain + self-loops, so message
    passing is a 3-tap stencil along the token axis. Edge scores come from
    folded projections (w_es = W_g1 @ a_src per head); softmax over <=3 taps
    uses taps-on-free-dim layout (no cross-partition ops); alpha is broadcast
    across feature partitions via a DRAM round-trip (SWDGE replicate DMA).
  - LSTM: input-to-hidden precomputed for all timesteps as matmuls
    (r-embedding as a rank-1 outer-product matmul; bias via a ones-row
    matmul). Recurrence keeps gates on partitions, batch 8 on free; sigmoid
    via tanh (host pre-scales i/f/o weights by 0.5) so one ACT op covers all
    gates; cell update uses fused scalar_tensor_tensor ops. States H~ = 2h,
    C~ = 2c absorb 0.5 factors (host folds 0.5 into W_hh and W_out rows).
"""
import sys
sys.path.insert(0, '/opt/trn_rl_repo')

from contextlib import ExitStack

import numpy as np
import ml_dtypes

import concourse.bass as bass
import concourse.bacc as bacc
import concourse.mybir as mybir
import concourse.tile as tile
from concourse import library_config
from concourse.bass_utils import run_bass_kernel_spmd

F32 = mybir.dt.float32
BF16 = mybir.dt.bfloat16
I16 = mybir.dt.int16
AF = mybir.ActivationFunctionType
ALU = mybir.AluOpType
BF = ml_dtypes.bfloat16

B, N, D = 64, 499, 256
NCORES = 8
SEQ = 8            # sequences per core
NP = 512           # padded sequence length
T = SEQ * NP       # tokens per core (4096)
H1 = 8             # GAT1 heads
NEG = -1.0e9


def _wrap_idx(idx_flat):
    """[n] int16 -> [128, n//16] wrap for dma_gather (item i at
    [i % 16, i // 16], replicated to 128 partitions)."""
    w = idx_flat.reshape(-1, 16).T
    return np.tile(w, (8, 1)).copy()


def _grid_idx(arr_core):
    g = np.zeros((SEQ, NP), np.int64)
    g[:, :N] = arr_core
    return g.reshape(-1)


def _attention(nc, attp, dscr, es, ed, nh, uid):
    """Chain 3-tap softmax. es/ed [nh, NP] fp32. Returns DRAM scratch
    [3*nh, NP] bf16 with alpha rows (3*h + tap)."""
    E = attp.tile([nh, 3, NP], F32, tag="E")
    nc.vector.tensor_tensor(E[:, 0, 1:NP], es[:, 0:NP - 1], ed[:, 1:NP],
                            op=ALU.add)
    nc.vector.tensor_tensor(E[:, 1, :], es[:], ed[:], op=ALU.add)
    nc.vector.tensor_tensor(E[:, 2, 0:NP - 1], es[:, 1:NP], ed[:, 0:NP - 1],
                            op=ALU.add)
    nc.vector.memset(E[:, 0, 0:1], NEG)
    nc.vector.memset(E[:, 0, 498:499], NEG)
    nc.vector.memset(E[:, 2, 497:NP], NEG)
    Ew = E[:].rearrange("p a b -> p (a b)")
    nc.vector.scalar_tensor_tensor(Ew, Ew, 0.2, Ew, ALU.mult, ALU.max)
    EX = attp.tile([nh, 3, NP], F32, tag="EX")
    nc.scalar.activation(EX[:].rearrange("p a b -> p (a b)"), Ew, AF.Exp)
    S = attp.tile([nh, NP], F32, tag="S")
    nc.vector.tensor_tensor(S[:], EX[:, 0, :], EX[:, 1, :], op=ALU.add)
    nc.vector.tensor_tensor(S[:], S[:], EX[:, 2, :], op=ALU.add)
    RS = attp.tile([nh, 1, NP], F32, tag="RS")
    nc.vector.reciprocal(RS[:, 0, :], S[:])
    AL = attp.tile([nh, 3, NP], BF16, tag="AL")
    nc.vector.tensor_tensor(AL[:], EX[:], RS[:].to_broadcast([nh, 3, NP]),
                            op=ALU.mult)
    scr = dscr.tile([3 * nh, NP], BF16, tag=f"scr{uid}")
    nc.sync.dma_start(scr[:].rearrange("(h t) f -> h t f", t=3), AL[:])
    return scr


def _repl_alpha(nc, albp, scr, m):
    alb = albp.tile([128, 3, NP], BF16, tag="alb")
    nc.gpsimd.dma_start(
        out=alb[:],
        in_=scr[3 * m:3 * m + 3, :].unsqueeze(0).to_broadcast([128, 3, NP]))
    return alb


def _msg3tap(nc, pool, h, alb, msg_tag):
    """msg[n] = a_self[n]*h[n] + a_left[n]*h[n-1] + a_right[n]*h[n+1]."""
    msg = pool.tile([128, NP], F32, tag=msg_tag)
    tl = pool.tile([128, NP - 1], BF16, tag=msg_tag + "l")
    tr = pool.tile([128, NP - 1], BF16, tag=msg_tag + "r")
    nc.vector.tensor_tensor(msg[:], h[:], alb[:, 1, :], op=ALU.mult)
    nc.vector.tensor_tensor(tl[:], h[:, 0:NP - 1], alb[:, 0, 1:NP],
                            op=ALU.mult)
    nc.vector.tensor_tensor(tr[:], h[:, 1:NP], alb[:, 2, 0:NP - 1],
                            op=ALU.mult)
    nc.vector.tensor_tensor(msg[:, 1:NP], msg[:, 1:NP], tl[:], op=ALU.add)
    nc.vector.tensor_tensor(msg[:, 0:NP - 1], msg[:, 0:NP - 1], tr[:],
                            op=ALU.add)
    return msg


def build_nc(n_steps=N, n_seq=SEQ):
    nc = bacc.Bacc("TRN2", target_bir_lowering=False, debug=False,
                   num_devices=NCORES)

    # ---------------- DRAM inputs ----------------
    d_tbl_p = nc.dram_tensor("tbl_p", [10001, D], BF16, kind="ExternalInput")
    d_tbl_q = nc.dram_tensor("tbl_q", [2001, D], BF16, kind="ExternalInput")
    d_tbl_a = nc.dram_tensor("tbl_a", [11, D], BF16, kind="ExternalInput")
    d_idx = {}
    for nm in ("p", "q", "aff"):   # per-seq wrapped [128, SEQ*32]
        d_idx[nm] = nc.dram_tensor(f"idx_{nm}", [128, SEQ * (NP // 16)], I16,
                                   kind="ExternalInput")
    for nm in ("qn", "pn"):        # whole-grid wrapped [128, T//16]
        d_idx[nm] = nc.dram_tensor(f"idx_{nm}", [128, T // 16], I16,
                                   kind="ExternalInput")
    d_rrow = nc.dram_tensor("r_row", [1, T], BF16, kind="ExternalInput")
    d_wa1 = nc.dram_tensor("wa1", [D, D], BF16, kind="ExternalInput")
    d_wg1 = nc.dram_tensor("wg1", [D, 1024], BF16, kind="ExternalInput")
    d_wesd1 = nc.dram_tensor("wesd1", [D, 2 * H1], BF16, kind="ExternalInput")
    d_wg2 = nc.dram_tensor("wg2", [1024, D], BF16, kind="ExternalInput")
    d_a2 = nc.dram_tensor("a2", [D, 2], BF16, kind="ExternalInput")
    d_w1s = nc.dram_tensor("w1s", [D, 1024], BF16, kind="ExternalInput")
    d_w2s = nc.dram_tensor("w2s", [D, 1024], BF16, kind="ExternalInput")
    d_w4s = nc.dram_tensor("w4s", [D, 1024], BF16, kind="ExternalInput")
    d_whh = nc.dram_tensor("whh", [D, 1024], BF16, kind="ExternalInput")
    d_brow = nc.dram_tensor("bias_row", [1, 1024], BF16, kind="ExternalInput")
    d_rdir = nc.dram_tensor("rdir_row", [1, 1024], BF16, kind="ExternalInput")
    d_bg1 = nc.dram_tensor("bg1", [128, 8], F32, kind="ExternalInput")
    d_bg2 = nc.dram_tensor("bg2", [128, 2], F32, kind="ExternalInput")
    d_wout = nc.dram_tensor("wout3", [D, 3], BF16, kind="ExternalInput")
    d_bout = nc.dram_tensor("bout", [1, 1], F32, kind="ExternalInput")
    d_ident = nc.dram_tensor("ident", [128, 128], BF16, kind="ExternalInput")
    d_one1 = nc.dram_tensor("one1", [1, 1], BF16, kind="ExternalInput")
    d_y = nc.dram_tensor("y", [SEQ, N], F32, kind="ExternalOutput")

    with tile.TileContext(nc) as tc, ExitStack() as ctx:
        g = ctx.enter_context(tc.tile_pool(name="glob", bufs=1))
        dscr = ctx.enter_context(tc.tile_pool(name="dscr", bufs=1,
                                              space="DRAM"))

        nc.gpsimd.load_library(library_config.mlp)

        def ld(dram, shape, dtype=BF16, tag=None):
            t_ = g.tile(shape, dtype, tag=tag)
            nc.sync.dma_start(t_[:], dram[:])
            return t_

        WA1 = ld(d_wa1[:].rearrange("(a k) m -> k a m", k=128), [128, 2, D], tag="wa1")
        WG1 = ld(d_wg1[:].rearrange("(a k) m -> k a m", k=128), [128, 2, 1024], tag="wg1")
        WESD1 = ld(d_wesd1[:].rearrange("(a k) m -> k a m", k=128), [128, 2, 2 * H1], tag="wesd1")
        WG2 = ld(d_wg2[:].rearrange("(a k) m -> k a m", k=128), [128, 8, D], tag="wg2")
        A2 = ld(d_a2[:].rearrange("(a k) m -> k a m", k=128), [128, 2, 2], tag="a2")
        W1S = ld(d_w1s[:].rearrange("(a k) m -> k a m", k=128), [128, 2, 1024], tag="w1s")
        W2S = ld(d_w2s[:].rearrange("(a k) m -> k a m", k=128), [128, 2, 1024], tag="w2s")
        W4S = ld(d_w4s[:].rearrange("(a k) m -> k a m", k=128), [128, 2, 1024], tag="w4s")
        WHH = ld(d_whh[:].rearrange("(a k) m -> k a m", k=128), [128, 2, 1024], tag="whh")
        BROW = ld(d_brow, [1, 1024], tag="brow")
        RDIR = ld(d_rdir, [1, 1024], tag="rdir")
        BG1 = ld(d_bg1, [128, 8], F32, tag="bg1")
        BG2 = ld(d_bg2, [128, 2], F32, tag="bg2")
        WOUT = ld(d_wout[:].rearrange("(a k) m -> k a m", k=128), [128, 2, 3], tag="wout")
        BOUT = ld(d_bout, [1, 1], F32, tag="bout")
        IDENT = ld(d_ident, [128, 128], tag="ident")
        ONE1 = ld(d_one1, [1, 1], tag="one1")
        IDX = {nm: ld(d_idx[nm], [128, d_idx[nm].shape[1]], I16, tag=f"idx{nm}")
               for nm in ("p", "q", "aff", "qn", "pn")}
        ONES = g.tile([1, NP], BF16)
        nc.vector.memset(ONES[:], 1.0)

        PRE = g.tile([128, 8, SEQ, NP], BF16)   # LSTM input precompute
        HS = g.tile([128, 2, SEQ, NP], BF16)    # H~ history
        nc.vector.memset(HS[:], 0.0)
        PQP = g.tile([1, SEQ, NP], BF16)        # qn/pn output partial

        # ============ per-sequence pre-LSTM pipeline ============
        with tc.tile_pool(name="sq2", bufs=2) as sq2, \
             tc.tile_pool(name="sq1", bufs=1) as sq1, \
             tc.tile_pool(name="albp", bufs=2) as albp, \
             tc.tile_pool(name="attp", bufs=1) as attp, \
             tc.tile_pool(name="ps", bufs=4, space="PSUM") as ps, \
             tc.tile_pool(name="pss", bufs=1, space="PSUM") as pss, \
             tc.tile_pool(name="pqs", bufs=2, space="PSUM") as pqs:

            for s in range(n_seq):
                c0 = s * NP
                i0 = s * (NP // 16)

                # --- gathers for this sequence ---
                PT = sq2.tile([128, 2, NP], BF16, tag="PT")
                nc.gpsimd.dma_gather(PT[:], d_tbl_p[:],
                                     IDX["p"][:, i0:i0 + NP // 16],
                                     NP, NP, D, transpose=True)
                QT = sq2.tile([128, 2, NP], BF16, tag="QT")
                nc.gpsimd.dma_gather(QT[:], d_tbl_q[:],
                                     IDX["q"][:, i0:i0 + NP // 16],
                                     NP, NP, D, transpose=True)
                AT = sq1.tile([128, 2, NP], BF16, tag="AT")
                nc.gpsimd.dma_gather(AT[:], d_tbl_a[:],
                                     IDX["aff"][:, i0:i0 + NP // 16],
                                     NP, NP, D, transpose=True)
                RG = sq2.tile([1, NP], BF16, tag="RG")
                nc.sync.dma_start(RG[:], d_rrow[:, c0:c0 + NP])

                # --- affcat: X = p_emb @ Wa1 + Aproj[aff] ---
                XT = sq1.tile([128, 2, NP], BF16, tag="XT")
                for m in range(2):
                    pm = ps.tile([128, NP], F32)
                    for k in range(2):
                        nc.tensor.matmul(pm[:],
                                         WA1[:, k, m * 128:(m + 1) * 128],
                                         PT[:, k, :],
                                         start=(k == 0), stop=(k == 1))
                    nc.vector.tensor_tensor(XT[:, m, :], pm[:], AT[:, m, :],
                                            op=ALU.add)

                # --- GAT1 ---
                h1 = sq1.tile([128, H1, NP], BF16, tag="h1")
                for m in range(H1):
                    pm = ps.tile([128, NP], F32)
                    for k in range(2):
                        nc.tensor.matmul(pm[:],
                                         WG1[:, k, m * 128:(m + 1) * 128],
                                         XT[:, k, :],
                                         start=(k == 0), stop=(k == 1))
                    nc.vector.tensor_copy(h1[:, m, :], pm[:])
                pes = pss.tile([H1, NP], F32, tag="pes")
                for k in range(2):
                    nc.tensor.matmul(pes[:], WESD1[:, k, 0:H1], XT[:, k, :],
                                     start=(k == 0), stop=(k == 1))
                ped = pss.tile([H1, NP], F32, tag="ped")
                for k in range(2):
                    nc.tensor.matmul(ped[:], WESD1[:, k, H1:2 * H1],
                                     XT[:, k, :], start=(k == 0), stop=(k == 1))
                ed1 = attp.tile([H1, NP], F32, tag="ed")
                nc.vector.tensor_copy(ed1[:], ped[:])

                scr1 = _attention(nc, attp, dscr, pes, ed1, H1, f"a{s}")

                # --- messages + ELU -> x1 ---
                x1 = sq1.tile([128, H1, NP], BF16, tag="x1")
                for m in range(H1):
                    alb = _repl_alpha(nc, albp, scr1, m)
                    msg = _msg3tap(nc, sq1, h1[:, m, :], alb, "ms")
                    rz = sq2.tile([128, NP], BF16, tag="rz")
                    mz = sq2.tile([128, NP], BF16, tag="mz")
                    nc.vector.tensor_scalar(rz[:], msg[:], BG1[:, m:m + 1],
                                            0.0, ALU.add, ALU.max)
                    nc.vector.tensor_scalar(mz[:], msg[:], BG1[:, m:m + 1],
                                            0.0, ALU.add, ALU.min)
                    et = sq2.tile([128, NP], BF16, tag="et")
                    nc.scalar.activation(et[:], mz[:], AF.Exp)
                    nc.vector.scalar_tensor_tensor(x1[:, m, :], rz[:], -1.0,
                                                   et[:], ALU.add, ALU.add)

                # --- GAT2 ---
                h2 = sq1.tile([128, 2, NP], BF16, tag="h2")
                for m in range(2):
                    pm = ps.tile([128, NP], F32)
                    for k in range(8):
                        nc.tensor.matmul(pm[:],
                                         WG2[:, k, m * 128:(m + 1) * 128],
                                         x1[:, k, :],
                                         start=(k == 0), stop=(k == 7))
                    nc.vector.tensor_copy(h2[:, m, :], pm[:])
                pes2 = pss.tile([1, NP], F32, tag="pes")
                for k in range(2):
                    nc.tensor.matmul(pes2[:], A2[:, k, 0:1], h2[:, k, :],
                                     start=(k == 0), stop=(k == 1))
                ped2 = pss.tile([1, NP], F32, tag="ped")
                for k in range(2):
                    nc.tensor.matmul(ped2[:], A2[:, k, 1:2], h2[:, k, :],
                                     start=(k == 0), stop=(k == 1))
                ed2 = attp.tile([1, NP], F32, tag="ed")
                nc.vector.tensor_copy(ed2[:], ped2[:])

                scr2 = _attention(nc, attp, dscr, pes2, ed2, 1, f"b{s}")
                alb2 = _repl_alpha(nc, albp, scr2, 0)

                X2T = sq1.tile([128, 2, NP], BF16, tag="X2T")
                for m in range(2):
                    msg = _msg3tap(nc, sq1, h2[:, m, :], alb2, "m2")
                    nc.vector.tensor_scalar(X2T[:, m, :], msg[:],
                                            BG2[:, m:m + 1], None, ALU.add)

                # --- LSTM input precompute ---
                for m in range(8):
                    pm = ps.tile([128, NP], F32)
                    mm = 0
                    for W_, src in ((W1S, PT), (W2S, QT), (W4S, X2T)):
                        for k in range(2):
                            nc.tensor.matmul(
                                pm[:], W_[:, k, m * 128:(m + 1) * 128],
                                src[:, k, :], start=(mm == 0), stop=False)
                            mm += 1
                    nc.tensor.matmul(pm[:], BROW[:, m * 128:(m + 1) * 128],
                                     ONES[:], start=False, stop=False)
                    nc.tensor.matmul(pm[:], RDIR[:, m * 128:(m + 1) * 128],
                                     RG[:], start=False, stop=True)
                    nc.vector.tensor_copy(PRE[:, m, s, :], pm[:])

                # --- qn/pn gathers + output partial (overlaps with LSTM) ---
                QNB = sq2.tile([128, 2, NP], BF16, tag="QNB")
                nc.gpsimd.dma_gather(QNB[:], d_tbl_q[:],
                                     IDX["qn"][:, i0:i0 + NP // 16],
                                     NP, NP, D, transpose=True)
                PNB = sq2.tile([128, 2, NP], BF16, tag="PNB")
                nc.gpsimd.dma_gather(PNB[:], d_tbl_p[:],
                                     IDX["pn"][:, i0:i0 + NP // 16],
                                     NP, NP, D, transpose=True)
                pq = pqs.tile([1, NP], F32, tag="pq")
                nc.tensor.matmul(pq[:], WOUT[:, 0, 1:2], QNB[:, 0, :],
                                 start=True, stop=False)
                nc.tensor.matmul(pq[:], WOUT[:, 1, 1:2], QNB[:, 1, :],
                                 start=False, stop=False)
                nc.tensor.matmul(pq[:], WOUT[:, 0, 2:3], PNB[:, 0, :],
                                 start=False, stop=False)
                nc.tensor.matmul(pq[:], WOUT[:, 1, 2:3], PNB[:, 1, :],
                                 start=False, stop=True)
                nc.vector.tensor_copy(PQP[0:1, s, :], pq[:])

        # ============ LSTM recurrence ============
        # Z layout (flat 96 cols): [f 0:16 | i 16:32 | C~ 32:48 | g 48:64 |
        # o 64:80 | pad]. PRE is injected into PSUM by an identity matmul so
        # the gate tanh reads PSUM directly; (f,i) gate math fuses into one
        # op against the contiguous [C~|g] block.
        with tc.tile_pool(name="lstm", bufs=2) as lp, \
             tc.tile_pool(name="lps", bufs=2, space="PSUM") as lps:
            Zt = g.tile([128, 2, 48], F32, tag="Zt")
            Zf = Zt[:].rearrange("p a b -> p (a b)")
            nc.vector.memset(Zf[:, 32:48], 0.0)
            for n in range(n_steps):
                pg = lps.tile([128, 64], F32)
                nc.tensor.matmul(pg[:], IDENT[:], PRE[:, :, :, n],
                                 start=True, stop=(n == 0),
                                 skip_group_check=True)
                if n > 0:
                    for j in range(8):
                        for kk in range(2):
                            nc.tensor.matmul(
                                pg[:, j * 8:(j + 1) * 8],
                                WHH[:, kk, j * 128:(j + 1) * 128],
                                HS[:, kk, :, n - 1],
                                start=False,
                                stop=(j == 7 and kk == 1),
                                skip_group_check=True)
                nc.scalar.activation(
                    Zt[:, :, 0:32],
                    pg[:].rearrange("p (a b) -> p a b", a=2),
                    AF.Tanh)
                U = lp.tile([128, 32], F32, tag="U")
                nc.vector.scalar_tensor_tensor(
                    U[:], Zt[:, 0, 0:32], 1.0, Zf[:, 32:64],
                    ALU.add, ALU.mult)
                nc.vector.scalar_tensor_tensor(
                    Zf[:, 32:48], U[:, 0:16], 0.5, U[:, 16:32],
                    ALU.mult, ALU.add)
                TCt = lp.tile([128, 16], F32, tag="TC")
                nc.scalar.activation(TCt[:], Zf[:, 32:48], AF.Tanh, scale=0.5)
                nc.vector.scalar_tensor_tensor(
                    HS[:, :, :, n], Zf[:, 64:80], 1.0, TCt[:],
                    ALU.add, ALU.mult)

        # ============ output (qn/pn part precomputed into PQP) ============
        with tc.tile_pool(name="outp", bufs=2) as op_, \
             tc.tile_pool(name="ops", bufs=2, space="PSUM") as ops_:
            for s in range(n_seq):
                py = ops_.tile([1, NP], F32)
                nc.tensor.matmul(py[:], WOUT[:, 0, 0:1], HS[:, 0, s, :],
                                 start=True, stop=False)
                nc.tensor.matmul(py[:], WOUT[:, 1, 0:1], HS[:, 1, s, :],
                                 start=False, stop=False)
                nc.tensor.matmul(py[:], ONE1[:], PQP[0:1, s, :],
                                 start=False, stop=True)
                ys = op_.tile([1, NP], F32, tag="ys")
                nc.scalar.activation(ys[:], py[:], AF.Sigmoid, bias=BOUT[:])
                nc.sync.dma_start(d_y[s, :], ys[0:1, 0:N])

    nc.compile()
    return nc


def _prep_inputs(inputs):
    f32 = lambda k: np.asarray(inputs[k], np.float32)
    emb_p, emb_q = f32('emb_p'), f32('emb_q')
    emb_r, emb_aff = f32('emb_r'), f32('emb_aff')
    W_affcat, b_affcat = f32('W_affcat'), f32('b_affcat')
    W_g1, a_src1, a_dst1, b_g1 = (f32('W_g1'), f32('a_src1'), f32('a_dst1'),
                                  f32('b_g1'))
    W_g2, a_src2, a_dst2, b_g2 = (f32('W_g2'), f32('a_src2'), f32('a_dst2'),
                                  f32('b_g2'))
    W_ih, W_hh, b_ih, b_hh = (f32('W_ih'), f32('W_hh'), f32('b_ih'),
                              f32('b_hh'))
    W_out, b_out = f32('W_out'), f32('b_out')

    Aproj = emb_aff @ W_affcat[D:] + b_affcat
    Wg1r = W_g1.reshape(D, H1, 128)
    w_es1 = np.einsum('dhf,hf->dh', Wg1r, a_src1)
    w_ed1 = np.einsum('dhf,hf->dh', Wg1r, a_dst1)
    wesd1 = np.concatenate([w_es1, w_ed1], axis=1)
    a2 = np.stack([a_src2[0], a_dst2[0]], axis=1)

    # gate-row permutation: torch (i,f,g,o) -> kernel (f,i,g,o) so the cell
    # update can fuse (f,i) ops and keep the C state adjacent to g.
    perm = np.concatenate([np.arange(D, 2 * D), np.arange(0, D),
                           np.arange(2 * D, 3 * D), np.arange(3 * D, 4 * D)])
    gs = np.ones((4 * D, 1), np.float32)
    gs[0:D] = 0.5; gs[D:2 * D] = 0.5; gs[3 * D:] = 0.5
    gs = gs[perm]
    W1s = (W_ih[perm, 0:D] * gs).T
    W2s = (W_ih[perm, D:2 * D] * gs).T
    W3 = W_ih[perm, 2 * D:3 * D]
    W4s = (W_ih[perm, 3 * D:4 * D] * gs).T
    bias_comb = (b_ih + b_hh)[perm] * gs[:, 0] + (emb_r[0] @ W3.T) * gs[:, 0]
    r_dir = ((emb_r[1] - emb_r[0]) @ W3.T) * gs[:, 0]
    W_hh_s = (W_hh[perm] * gs * 0.5).T
    wout3 = np.stack([W_out[0:D, 0] * 0.5, W_out[D:2 * D, 0],
                      W_out[2 * D:3 * D, 0]], axis=1)

    shared = {
        'tbl_p': emb_p.astype(BF), 'tbl_q': emb_q.astype(BF),
        'tbl_a': Aproj.astype(BF),
        'wa1': W_affcat[:D].astype(BF),
        'wg1': W_g1.astype(BF), 'wesd1': wesd1.astype(BF),
        'wg2': W_g2.astype(BF), 'a2': a2.astype(BF),
        'w1s': W1s.astype(BF), 'w2s': W2s.astype(BF), 'w4s': W4s.astype(BF),
        'whh': W_hh_s.astype(BF),
        'bias_row': bias_comb[None, :].astype(BF),
        'rdir_row': r_dir[None, :].astype(BF),
        'bg1': b_g1.reshape(8, 128).T.copy().astype(np.float32),
        'bg2': b_g2.reshape(2, 128).T.copy().astype(np.float32),
        'wout3': wout3.astype(BF),
        'bout': b_out.reshape(1, 1).astype(np.float32),
        'ident': np.eye(128, dtype=np.float32).astype(BF),
        'one1': np.ones((1, 1), np.float32).astype(BF),
    }

    p = np.asarray(inputs['p']); q = np.asarray(inputs['q'])
    r = np.asarray(inputs['r']); aff = np.asarray(inputs['aff'])
    q_next = np.asarray(inputs['q_next']); p_next = np.asarray(inputs['p_next'])

    def per_seq_wrap(arr_core):
        grid = np.zeros((SEQ, NP), np.int64)
        grid[:, :N] = arr_core
        cols = [_wrap_idx(grid[s].astype(np.int16)) for s in range(SEQ)]
        return np.concatenate(cols, axis=1)  # [128, SEQ*32]

    in_maps = []
    for c in range(NCORES):
        sl = slice(c * SEQ, (c + 1) * SEQ)
        m = dict(shared)
        m['idx_p'] = per_seq_wrap(p[sl])
        m['idx_q'] = per_seq_wrap(q[sl])
        m['idx_aff'] = per_seq_wrap(aff[sl])
        m['idx_qn'] = per_seq_wrap(q_next[sl])
        m['idx_pn'] = per_seq_wrap(p_next[sl])
        rg = np.zeros((SEQ, NP), np.float32)
        rg[:, :N] = r[sl]
        m['r_row'] = rg.reshape(1, T).astype(BF)
        in_maps.append(m)
    return in_maps


_NC_CACHE = {}
TRACE = False
LAST_RESULT = None


def kernel(**inputs):
    global LAST_RESULT
    in_maps = _prep_inputs(inputs)
    if 'nc' not in _NC_CACHE:
        _NC_CACHE['nc'] = build_nc()
    nc = _NC_CACHE['nc']
    res = run_bass_kernel_spmd(nc, in_maps, core_ids=list(range(NCORES)),
                               trace=TRACE)
    LAST_RESULT = res
    y = np.concatenate([res.results[c]['y'] for c in range(NCORES)], axis=0)
    return y.reshape(B, N, 1).astype(np.float32)


if __name__ == "__main__":
    data = np.load('/root/problem/work/inputs.npz')
    inp = {k: data[k] for k in data.files}
    y = kernel(**inp)
    exp = np.load('/root/problem/work/expected.npy')
    err = np.abs(y - exp).max()
    print("max abs err:", err, "rel:", err / np.abs(exp).max())



# revision 27
# speedup vs baseline: 1.4713x; 1.4713x over previous
"""Trainium2 Bass kernel for nn_DKT (GAT chain-graph + LSTM network).

Strategy: data-parallel over batch (8 sequences per core x 8 cores).
Per core, tokens live on a padded grid t = s*512 + n (n < 499 real).
All activations are feature-major ([feature-partition, token-free]) so every
dense layer is a PE matmul with bf16 operands and fp32 PSUM accumulation.

v3: the GAT/embedding pre-computation is token-blocked (9 blocks of 56 real
tokens + 4-token halo, all 8 sequences per block) and its instruction
emission is interleaved with the LSTM recurrence steps, so the pre-work for
block b executes inside the engine-idle windows of LSTM steps for blocks
b-2/b-1. The qn/pn output contributions are also precomputed per block.
The LSTM step itself injects PRE into PSUM with an identity matmul (gate
tanh reads PSUM directly) and fuses the cell update via a
[f | i | C~ | g | o] column layout.
"""
import sys
sys.path.insert(0, '/opt/trn_rl_repo')

from contextlib import ExitStack

import numpy as np
import ml_dtypes

import concourse.bass as bass
import concourse.bacc as bacc
import concourse.mybir as mybir
import concourse.tile as tile
from concourse import library_config
from concourse.bass_utils import run_bass_kernel_spmd

F32 = mybir.dt.float32
BF16 = mybir.dt.bfloat16
I16 = mybir.dt.int16
AF = mybir.ActivationFunctionType
ALU = mybir.AluOpType
BF = ml_dtypes.bfloat16

B, N, D = 64, 499, 256
NCORES = 8
SEQ = 8            # sequences per core
NP = 512           # padded sequence length
H1 = 8             # GAT1 heads
NEG = -1.0e9
NB, BW, BC, HL = 9, 64, 56, 4   # blocks, block width, central, halo


def _wrap_idx(idx_flat):
    """[n] int16 -> [128, n//16] wrap for dma_gather (item i at
    [i % 16, i // 16], replicated to 128 partitions)."""
    w = idx_flat.reshape(-1, 16).T
    return np.tile(w, (8, 1)).copy()


def build_nc(n_steps=N):
    nc = bacc.Bacc("TRN2", target_bir_lowering=False, debug=False,
                   num_devices=NCORES)

    # ---------------- DRAM inputs ----------------
    d_tbl_p = nc.dram_tensor("tbl_p", [10001, D], BF16, kind="ExternalInput")
    d_tbl_q = nc.dram_tensor("tbl_q", [2001, D], BF16, kind="ExternalInput")
    d_tbl_a = nc.dram_tensor("tbl_a", [11, D], BF16, kind="ExternalInput")
    d_idx = {}
    for nm in ("p", "q", "aff", "qn", "pn"):   # blocked wrapped [128, NB*32]
        d_idx[nm] = nc.dram_tensor(f"idx_{nm}", [128, NB * (SEQ * BW // 16)],
                                   I16, kind="ExternalInput")
    d_rrow = nc.dram_tensor("r_row", [SEQ, BC * NB + 2 * HL + 8], BF16,
                            kind="ExternalInput")
    d_wa1 = nc.dram_tensor("wa1", [D, D], BF16, kind="ExternalInput")
    d_wg1 = nc.dram_tensor("wg1", [D, 1024], BF16, kind="ExternalInput")
    d_wesd1 = nc.dram_tensor("wesd1", [D, 2 * H1], BF16, kind="ExternalInput")
    d_wg2 = nc.dram_tensor("wg2", [1024, D], BF16, kind="ExternalInput")
    d_a2 = nc.dram_tensor("a2", [D, 2], BF16, kind="ExternalInput")
    d_w1s = nc.dram_tensor("w1s", [D, 1024], BF16, kind="ExternalInput")
    d_w2s = nc.dram_tensor("w2s", [D, 1024], BF16, kind="ExternalInput")
    d_w4s = nc.dram_tensor("w4s", [D, 1024], BF16, kind="ExternalInput")
    d_whh = nc.dram_tensor("whh", [D, 1024], BF16, kind="ExternalInput")
    d_rdir = nc.dram_tensor("rdir_row", [1, 1024], BF16, kind="ExternalInput")
    d_biasp = nc.dram_tensor("biasp", [128, 8], F32, kind="ExternalInput")
    d_bg1 = nc.dram_tensor("bg1", [128, 8], F32, kind="ExternalInput")
    d_bg2 = nc.dram_tensor("bg2", [128, 2], F32, kind="ExternalInput")
    d_wout = nc.dram_tensor("wout3", [D, 3], BF16, kind="ExternalInput")
    d_bout = nc.dram_tensor("bout", [1, 1], F32, kind="ExternalInput")
    d_ident = nc.dram_tensor("ident", [128, 128], BF16, kind="ExternalInput")
    d_one1 = nc.dram_tensor("one1", [1, 1], BF16, kind="ExternalInput")
    d_y = nc.dram_tensor("y", [SEQ, N], F32, kind="ExternalOutput")

    with tile.TileContext(nc) as tc, ExitStack() as ctx:
        g = ctx.enter_context(tc.tile_pool(name="glob", bufs=1))
        dscr = ctx.enter_context(tc.tile_pool(name="dscr", bufs=1,
                                              space="DRAM"))
        bp2 = ctx.enter_context(tc.tile_pool(name="bp2", bufs=2))
        bp1 = ctx.enter_context(tc.tile_pool(name="bp1", bufs=1))
        attp = ctx.enter_context(tc.tile_pool(name="attp", bufs=1))
        albp = ctx.enter_context(tc.tile_pool(name="albp", bufs=2))
        ps = ctx.enter_context(tc.tile_pool(name="ps", bufs=2, space="PSUM"))
        pss = ctx.enter_context(tc.tile_pool(name="pss", bufs=1,
                                             space="PSUM"))
        pqs = ctx.enter_context(tc.tile_pool(name="pqs", bufs=1,
                                             space="PSUM"))
        lps = ctx.enter_context(tc.tile_pool(name="lps", bufs=2,
                                             space="PSUM"))

        nc.gpsimd.load_library(library_config.mlp)

        def ld(dram, shape, dtype=BF16, tag=None):
            t_ = g.tile(shape, dtype, tag=tag)
            nc.sync.dma_start(t_[:], dram[:])
            return t_

        WA1 = ld(d_wa1[:].rearrange("(a k) m -> k a m", k=128), [128, 2, D], tag="wa1")
        WG1 = ld(d_wg1[:].rearrange("(a k) m -> k a m", k=128), [128, 2, 1024], tag="wg1")
        WESD1 = ld(d_wesd1[:].rearrange("(a k) m -> k a m", k=128), [128, 2, 2 * H1], tag="wesd1")
        WG2 = ld(d_wg2[:].rearrange("(a k) m -> k a m", k=128), [128, 8, D], tag="wg2")
        A2 = ld(d_a2[:].rearrange("(a k) m -> k a m", k=128), [128, 2, 2], tag="a2")
        W1S = ld(d_w1s[:].rearrange("(a k) m -> k a m", k=128), [128, 2, 1024], tag="w1s")
        W2S = ld(d_w2s[:].rearrange("(a k) m -> k a m", k=128), [128, 2, 1024], tag="w2s")
        W4S = ld(d_w4s[:].rearrange("(a k) m -> k a m", k=128), [128, 2, 1024], tag="w4s")
        WHH = ld(d_whh[:].rearrange("(a k) m -> k a m", k=128), [128, 2, 1024], tag="whh")
        RDIR = ld(d_rdir, [1, 1024], tag="rdir")
        BIASP = ld(d_biasp, [128, 8], F32, tag="biasp")
        BG1 = ld(d_bg1, [128, 8], F32, tag="bg1")
        BG2 = ld(d_bg2, [128, 2], F32, tag="bg2")
        WOUT = ld(d_wout[:].rearrange("(a k) m -> k a m", k=128), [128, 2, 3], tag="wout")
        BOUT = ld(d_bout, [1, 1], F32, tag="bout")
        IDENT = ld(d_ident, [128, 128], tag="ident")
        ONE1 = ld(d_one1, [1, 1], tag="one1")
        IDX = {nm: ld(d_idx[nm], [128, d_idx[nm].shape[1]], I16,
                      tag=f"idx{nm}")
               for nm in ("p", "q", "aff", "qn", "pn")}

        PRE = g.tile([128, 8, SEQ, NP], BF16)   # LSTM input precompute
        HS = g.tile([128, 2, SEQ, NP], BF16)    # H~ history
        nc.vector.memset(HS[:], 0.0)
        PQP = g.tile([1, SEQ, NP], BF16)        # qn/pn output partial

        NW = SEQ * BW        # 512 tokens per block (8 seqs x 64)
        ICOL = SEQ * BW // 16  # idx cols per block (32)

        def split_st(ap):
            return ap.rearrange("p (s t) -> p s t", s=SEQ)

        def attention_blk(bs, es, ed, nh, uid, masks):
            """Blocked 3-tap chain softmax; es PSUM [nh, NW], ed SBUF."""
            E = attp.tile([nh, 3, NW], F32, tag="E")
            e3 = [split_st(E[:, t, :]) for t in range(3)]
            ess, eds = split_st(es[:]), split_st(ed[:])
            nc.vector.tensor_tensor(e3[0][:, :, 1:BW], ess[:, :, 0:BW - 1],
                                    eds[:, :, 1:BW], op=ALU.add)
            nc.vector.tensor_tensor(E[:, 1, :], es[:], ed[:], op=ALU.add)
            nc.vector.tensor_tensor(e3[2][:, :, 0:BW - 1], ess[:, :, 1:BW],
                                    eds[:, :, 0:BW - 1], op=ALU.add)
            nc.vector.memset(e3[0][:, :, 0:1], NEG)
            nc.vector.memset(e3[2][:, :, BW - 1:BW], NEG)
            for tap, l in masks:
                nc.vector.memset(e3[tap][:, :, l:l + 1], NEG)
            Ew = E[:].rearrange("p a b -> p (a b)")
            nc.vector.scalar_tensor_tensor(Ew, Ew, 0.2, Ew, ALU.mult, ALU.max)
            EX = attp.tile([nh, 3, NW], BF16, tag="EX")
            nc.scalar.activation(EX[:].rearrange("p a b -> p (a b)"), Ew,
                                 AF.Exp)
            S = attp.tile([nh, NW], F32, tag="S")
            nc.vector.tensor_tensor(S[:], EX[:, 0, :], EX[:, 1, :],
                                    op=ALU.add)
            nc.vector.tensor_tensor(S[:], S[:], EX[:, 2, :], op=ALU.add)
            RS = attp.tile([nh, 1, NW], F32, tag="RS")
            nc.vector.reciprocal(RS[:, 0, :], S[:])
            AL = attp.tile([nh, 3, NW], BF16, tag="AL")
            nc.vector.tensor_tensor(AL[:], EX[:],
                                    RS[:].to_broadcast([nh, 3, NW]),
                                    op=ALU.mult)
            scr = dscr.tile([3 * nh, NW], BF16, tag=f"scr{uid}")
            nc.sync.dma_start(scr[:].rearrange("(h t) f -> h t f", t=3),
                              AL[:])
            return scr

        def repl_alpha(scr, m):
            alb = albp.tile([128, 3, NW], BF16, tag="alb")
            nc.gpsimd.dma_start(
                out=alb[:],
                in_=scr[3 * m:3 * m + 3, :].unsqueeze(0)
                .to_broadcast([128, 3, NW]))
            return alb

        def msg3tap(h, alb, msg_tag):
            """msg[t] = a_self[t]*h[t] + a_left[t]*h[t-1] + a_right[t]*h[t+1]
            per sequence (3D shifted APs, no cross-seq bleed)."""
            msg = bp2.tile([128, SEQ, BW], F32, tag=msg_tag, bufs=1)
            tl = bp2.tile([128, SEQ, BW - 1], BF16, tag=msg_tag + "l", bufs=1)
            tr = bp2.tile([128, SEQ, BW - 1], BF16, tag=msg_tag + "r", bufs=1)
            hs_, a0 = split_st(h), split_st(alb[:, 0, :])
            a1, a2_ = split_st(alb[:, 1, :]), split_st(alb[:, 2, :])
            nc.vector.tensor_tensor(msg[:].rearrange("p s t -> p (s t)"), h,
                                    alb[:, 1, :], op=ALU.mult)
            nc.vector.tensor_tensor(tl[:], hs_[:, :, 0:BW - 1],
                                    a0[:, :, 1:BW], op=ALU.mult)
            nc.vector.tensor_tensor(tr[:], hs_[:, :, 1:BW],
                                    a2_[:, :, 0:BW - 1], op=ALU.mult)
            nc.vector.tensor_tensor(msg[:, :, 1:BW], msg[:, :, 1:BW], tl[:],
                                    op=ALU.add)
            nc.vector.tensor_tensor(msg[:, :, 0:BW - 1], msg[:, :, 0:BW - 1],
                                    tr[:], op=ALU.add)
            return msg

        # ---------- per-block emission units ----------
        all_units = []

        def emit_block(b):
            bs = {}
            c0 = BC * b
            i0 = b * ICOL
            masks = []
            for g_, tap in ((0, 0), (498, 0), (497, 2), (498, 2)):
                l = g_ - (BC * b - HL)
                if 0 <= l < BW:
                    masks.append((tap, l))

            def u(fn):
                all_units.append((b, fn))

            def u_gather(nm, tbl, key):
                def fn():
                    t_ = bp2.tile([128, 2, NW], BF16, tag=key)
                    nc.gpsimd.dma_gather(t_[:], tbl[:],
                                         IDX[nm][:, i0:i0 + ICOL],
                                         NW, NW, D, transpose=True)
                    bs[key] = t_
                u(fn)

            u_gather("p", d_tbl_p, "PT")
            u_gather("aff", d_tbl_a, "AT")
            u_gather("q", d_tbl_q, "QT")
            u_gather("qn", d_tbl_q, "QN")
            u_gather("pn", d_tbl_p, "PN")

            def u_rg():
                rg = bp2.tile([1, SEQ, BW], BF16, tag="RG")
                nc.sync.dma_start(rg[:], d_rrow[:, c0:c0 + BW].unsqueeze(0))
                bs["RG"] = rg
            u(u_rg)

            def u_xt(m):
                def fn():
                    if "XT" not in bs:
                        bs["XT"] = bp1.tile([128, 2, NW], BF16, tag="XT", name="XT")
                    pm = ps.tile([128, NW], F32)
                    for k in range(2):
                        nc.tensor.matmul(pm[:],
                                         WA1[:, k, m * 128:(m + 1) * 128],
                                         bs["PT"][:, k, :],
                                         start=(k == 0), stop=(k == 1))
                    nc.vector.tensor_tensor(bs["XT"][:, m, :], pm[:],
                                            bs["AT"][:, m, :], op=ALU.add)
                return fn
            u(u_xt(0))
            u(u_xt(1))

            def u_esd1():
                pes = pss.tile([H1, NW], F32, tag="pes")
                for k in range(2):
                    nc.tensor.matmul(pes[:], WESD1[:, k, 0:H1],
                                     bs["XT"][:, k, :],
                                     start=(k == 0), stop=(k == 1))
                ped = pss.tile([H1, NW], F32, tag="ped")
                for k in range(2):
                    nc.tensor.matmul(ped[:], WESD1[:, k, H1:2 * H1],
                                     bs["XT"][:, k, :],
                                     start=(k == 0), stop=(k == 1))
                ed1 = attp.tile([H1, NW], F32, tag="ed")
                nc.vector.tensor_copy(ed1[:], ped[:])
                bs["pes"], bs["ed1"] = pes, ed1
            u(u_esd1)

            def u_h1(m):
                def fn():
                    if "h1" not in bs:
                        bs["h1"] = bp1.tile([128, H1, NW], BF16, tag="h1", name="h1")
                    pm = ps.tile([128, NW], F32)
                    for k in range(2):
                        nc.tensor.matmul(pm[:],
                                         WG1[:, k, m * 128:(m + 1) * 128],
                                         bs["XT"][:, k, :],
                                         start=(k == 0), stop=(k == 1))
                    nc.vector.tensor_copy(bs["h1"][:, m, :], pm[:])
                return fn
            for m in range(H1):
                u(u_h1(m))

            def u_att1():
                bs["scr1"] = attention_blk(bs, bs["pes"], bs["ed1"], H1,
                                           f"a{b}", masks)
            u(u_att1)

            def u_msg1(m):
                def fn():
                    if "x1" not in bs:
                        bs["x1"] = bp1.tile([128, H1, NW], BF16, tag="x1", name="x1")
                    alb = repl_alpha(bs["scr1"], m)
                    msg = msg3tap(bs["h1"][:, m, :], alb, "ms")
                    mf = msg[:].rearrange("p s t -> p (s t)")
                    rz = bp2.tile([128, NW], BF16, tag="rz", bufs=1)
                    mz = bp2.tile([128, NW], BF16, tag="mz", bufs=1)
                    nc.vector.tensor_scalar(rz[:], mf, BG1[:, m:m + 1],
                                            0.0, ALU.add, ALU.max)
                    nc.vector.tensor_scalar(mz[:], mf, BG1[:, m:m + 1],
                                            0.0, ALU.add, ALU.min)
                    et = bp2.tile([128, NW], BF16, tag="et", bufs=1)
                    nc.scalar.activation(et[:], mz[:], AF.Exp)
                    nc.vector.scalar_tensor_tensor(bs["x1"][:, m, :], rz[:],
                                                   -1.0, et[:], ALU.add,
                                                   ALU.add)
                return fn
            for m in range(H1):
                u(u_msg1(m))

            def u_h2(m):
                def fn():
                    if "h2" not in bs:
                        bs["h2"] = bp2.tile([128, 2, NW], BF16, tag="h2", name="h2")
                    pm = ps.tile([128, NW], F32)
                    for k in range(8):
                        nc.tensor.matmul(pm[:],
                                         WG2[:, k, m * 128:(m + 1) * 128],
                                         bs["x1"][:, k, :],
                                         start=(k == 0), stop=(k == 7))
                    nc.vector.tensor_copy(bs["h2"][:, m, :], pm[:])
                return fn
            u(u_h2(0))
            u(u_h2(1))

            def u_att2():
                pes2 = pss.tile([1, NW], F32, tag="pes")
                for k in range(2):
                    nc.tensor.matmul(pes2[:], A2[:, k, 0:1],
                                     bs["h2"][:, k, :],
                                     start=(k == 0), stop=(k == 1))
                ped2 = pss.tile([1, NW], F32, tag="ped")
                for k in range(2):
                    nc.tensor.matmul(ped2[:], A2[:, k, 1:2],
                                     bs["h2"][:, k, :],
                                     start=(k == 0), stop=(k == 1))
                ed2 = attp.tile([1, NW], F32, tag="ed")
                nc.vector.tensor_copy(ed2[:], ped2[:])
                bs["scr2"] = attention_blk(bs, pes2, ed2, 1, f"b{b}", masks)
            u(u_att2)

            def u_x2(m):
                def fn():
                    if "alb2" not in bs:
                        bs["alb2"] = repl_alpha(bs["scr2"], 0)
                    if "X2T" not in bs:
                        bs["X2T"] = bp2.tile([128, 2, NW], BF16, tag="X2T", name="X2T")
                    msg = msg3tap(bs["h2"][:, m, :], bs["alb2"], "m2")
                    nc.vector.tensor_scalar(bs["X2T"][:, m, :],
                                            msg[:].rearrange(
                                                "p s t -> p (s t)"),
                                            BG2[:, m:m + 1], None, ALU.add)
                return fn
            u(u_x2(0))
            u(u_x2(1))

            def u_pre(m):
                def fn():
                    pm = ps.tile([128, NW], F32)
                    mm = 0
                    for W_, key in ((W1S, "PT"), (W2S, "QT"), (W4S, "X2T")):
                        for k in range(2):
                            nc.tensor.matmul(
                                pm[:], W_[:, k, m * 128:(m + 1) * 128],
                                bs[key][:, k, :], start=(mm == 0),
                                stop=False)
                            mm += 1
                    nc.tensor.matmul(pm[:], RDIR[:, m * 128:(m + 1) * 128],
                                     bs["RG"][:].rearrange("p s t -> p (s t)"),
                                     start=False, stop=True)
                    nc.vector.tensor_scalar(
                        PRE[:, m, :, c0:c0 + BC],
                        split_st(pm[:])[:, :, HL:HL + BC],
                        BIASP[:, m:m + 1], None, ALU.add)
                return fn
            for m in range(8):
                u(u_pre(m))

            def u_pq():
                pq = pqs.tile([1, SEQ, BC], F32, tag="pq")
                nc.tensor.matmul(pq[:], WOUT[:, 0, 1:2],
                                 split_st(bs["QN"][:, 0, :])[:, :, HL:HL + BC],
                                 start=True, stop=False)
                nc.tensor.matmul(pq[:], WOUT[:, 1, 1:2],
                                 split_st(bs["QN"][:, 1, :])[:, :, HL:HL + BC],
                                 start=False, stop=False)
                nc.tensor.matmul(pq[:], WOUT[:, 0, 2:3],
                                 split_st(bs["PN"][:, 0, :])[:, :, HL:HL + BC],
                                 start=False, stop=False)
                nc.tensor.matmul(pq[:], WOUT[:, 1, 2:3],
                                 split_st(bs["PN"][:, 1, :])[:, :, HL:HL + BC],
                                 start=False, stop=True)
                nc.vector.tensor_copy(PQP[0:1, :, c0:c0 + BC], pq[:])
            u(u_pq)

        for b in range(NB):
            emit_block(b)

        cursor = [0]

        def pump(target_block, budget=None):
            n_done = 0
            while cursor[0] < len(all_units):
                blk, fn = all_units[cursor[0]]
                if blk > target_block:
                    break
                if budget is not None and n_done >= budget:
                    break
                fn()
                cursor[0] += 1
                n_done += 1

        # prologue: blocks 0 and 1 fully emitted before the recurrence
        pump(1)

        # ============ LSTM recurrence (pre-work interleaved) ============
        Zt = g.tile([128, 2, 48], F32, tag="Zt")
        Zf = Zt[:].rearrange("p a b -> p (a b)")
        nc.vector.memset(Zf[:, 32:48], 0.0)
        for n in range(n_steps):
            pg = lps.tile([128, 64], F32)
            nc.tensor.matmul(pg[:], IDENT[:], PRE[:, :, :, n],
                             start=True, stop=(n == 0),
                             skip_group_check=True)
            if n > 0:
                for kk in range(2):
                    for j in range(8):
                        nc.tensor.matmul(
                            pg[:, j * 8:(j + 1) * 8],
                            WHH[:, kk, j * 128:(j + 1) * 128],
                            HS[:, kk, :, n - 1],
                            start=False,
                            stop=(kk == 1 and j == 7),
                            skip_group_check=True)
            nc.scalar.activation(
                Zt[:, :, 0:32],
                pg[:].rearrange("p (a b) -> p a b", a=2),
                AF.Tanh)
            U = bp2.tile([128, 32], F32, tag="U")
            nc.vector.scalar_tensor_tensor(
                U[:], Zt[:, 0, 0:32], 1.0, Zf[:, 32:64],
                ALU.add, ALU.mult)
            nc.vector.scalar_tensor_tensor(
                Zf[:, 32:48], U[:, 0:16], 0.5, U[:, 16:32],
                ALU.mult, ALU.add)
            TCt = bp2.tile([128, 16], F32, tag="TC")
            nc.scalar.activation(TCt[:], Zf[:, 32:48], AF.Tanh, scale=0.5)
            nc.vector.scalar_tensor_tensor(
                HS[:, 0, :, n], Zf[:, 64:72], 1.0, TCt[:, 0:8],
                ALU.add, ALU.mult)
            nc.vector.scalar_tensor_tensor(
                HS[:, 1, :, n], Zf[:, 72:80], 1.0, TCt[:, 8:16],
                ALU.add, ALU.mult)
            pump(n // BC + 2, budget=3)

        pump(NB)

        # ============ output (qn/pn part precomputed into PQP) ============
        for s in range(SEQ):
            py = pqs.tile([1, NP], F32, tag="pq")
            nc.tensor.matmul(py[:], WOUT[:, 0, 0:1], HS[:, 0, s, :],
                             start=True, stop=False)
            nc.tensor.matmul(py[:], WOUT[:, 1, 0:1], HS[:, 1, s, :],
                             start=False, stop=False)
            nc.tensor.matmul(py[:], ONE1[:], PQP[0:1, s, :],
                             start=False, stop=True)
            ys = bp2.tile([1, NP], F32, tag="ys")
            nc.scalar.activation(ys[:], py[:], AF.Sigmoid, bias=BOUT[:])
            nc.sync.dma_start(d_y[s, :], ys[0:1, 0:N])

    nc.compile()
    return nc


def _prep_inputs(inputs):
    f32 = lambda k: np.asarray(inputs[k], np.float32)
    emb_p, emb_q = f32('emb_p'), f32('emb_q')
    emb_r, emb_aff = f32('emb_r'), f32('emb_aff')
    W_affcat, b_affcat = f32('W_affcat'), f32('b_affcat')
    W_g1, a_src1, a_dst1, b_g1 = (f32('W_g1'), f32('a_src1'), f32('a_dst1'),
                                  f32('b_g1'))
    W_g2, a_src2, a_dst2, b_g2 = (f32('W_g2'), f32('a_src2'), f32('a_dst2'),
                                  f32('b_g2'))
    W_ih, W_hh, b_ih, b_hh = (f32('W_ih'), f32('W_hh'), f32('b_ih'),
                              f32('b_hh'))
    W_out, b_out = f32('W_out'), f32('b_out')

    Aproj = emb_aff @ W_affcat[D:] + b_affcat
    Wg1r = W_g1.reshape(D, H1, 128)
    w_es1 = np.einsum('dhf,hf->dh', Wg1r, a_src1)
    w_ed1 = np.einsum('dhf,hf->dh', Wg1r, a_dst1)
    wesd1 = np.concatenate([w_es1, w_ed1], axis=1)
    a2 = np.stack([a_src2[0], a_dst2[0]], axis=1)

    # gate-row permutation: torch (i,f,g,o) -> kernel (f,i,g,o) so the cell
    # update can fuse (f,i) ops and keep the C state adjacent to g.
    perm = np.concatenate([np.arange(D, 2 * D), np.arange(0, D),
                           np.arange(2 * D, 3 * D), np.arange(3 * D, 4 * D)])
    gs = np.ones((4 * D, 1), np.float32)
    gs[0:D] = 0.5; gs[D:2 * D] = 0.5; gs[3 * D:] = 0.5
    gs = gs[perm]
    W1s = (W_ih[perm, 0:D] * gs).T
    W2s = (W_ih[perm, D:2 * D] * gs).T
    W3 = W_ih[perm, 2 * D:3 * D]
    W4s = (W_ih[perm, 3 * D:4 * D] * gs).T
    bias_comb = (b_ih + b_hh)[perm] * gs[:, 0] + (emb_r[0] @ W3.T) * gs[:, 0]
    r_dir = ((emb_r[1] - emb_r[0]) @ W3.T) * gs[:, 0]
    W_hh_s = (W_hh[perm] * gs * 0.5).T
    wout3 = np.stack([W_out[0:D, 0] * 0.5, W_out[D:2 * D, 0],
                      W_out[2 * D:3 * D, 0]], axis=1)

    shared = {
        'tbl_p': emb_p.astype(BF), 'tbl_q': emb_q.astype(BF),
        'tbl_a': Aproj.astype(BF),
        'wa1': W_affcat[:D].astype(BF),
        'wg1': W_g1.astype(BF), 'wesd1': wesd1.astype(BF),
        'wg2': W_g2.astype(BF), 'a2': a2.astype(BF),
        'w1s': W1s.astype(BF), 'w2s': W2s.astype(BF), 'w4s': W4s.astype(BF),
        'whh': W_hh_s.astype(BF),
        'rdir_row': r_dir[None, :].astype(BF),
        'biasp': bias_comb.reshape(8, 128).T.copy().astype(np.float32),
        'bg1': b_g1.reshape(8, 128).T.copy().astype(np.float32),
        'bg2': b_g2.reshape(2, 128).T.copy().astype(np.float32),
        'wout3': wout3.astype(BF),
        'bout': b_out.reshape(1, 1).astype(np.float32),
        'ident': np.eye(128, dtype=np.float32).astype(BF),
        'one1': np.ones((1, 1), np.float32).astype(BF),
    }

    p = np.asarray(inputs['p']); q = np.asarray(inputs['q'])
    r = np.asarray(inputs['r']); aff = np.asarray(inputs['aff'])
    q_next = np.asarray(inputs['q_next']); p_next = np.asarray(inputs['p_next'])

    def blocked_wrap(arr_core):
        grid = np.zeros((SEQ, NP), np.int64)
        grid[:, :N] = arr_core
        cols = []
        for b in range(NB):
            blk = np.zeros((SEQ, BW), np.int64)
            g0 = BC * b - HL
            lo, hi = max(0, -g0), min(BW, NP - g0)
            blk[:, lo:hi] = grid[:, g0 + lo:g0 + hi]
            cols.append(_wrap_idx(blk.reshape(-1).astype(np.int16)))
        return np.concatenate(cols, axis=1)  # [128, NB*32]

    RW = BC * NB + 2 * HL + 8
    in_maps = []
    for c in range(NCORES):
        sl = slice(c * SEQ, (c + 1) * SEQ)
        m = dict(shared)
        m['idx_p'] = blocked_wrap(p[sl])
        m['idx_q'] = blocked_wrap(q[sl])
        m['idx_aff'] = blocked_wrap(aff[sl])
        m['idx_qn'] = blocked_wrap(q_next[sl])
        m['idx_pn'] = blocked_wrap(p_next[sl])
        rg = np.zeros((SEQ, RW), np.float32)
        rg[:, HL:HL + N] = r[sl]
        m['r_row'] = rg.astype(BF)
        in_maps.append(m)
    return in_maps


_NC_CACHE = {}
TRACE = False
LAST_RESULT = None


def kernel(**inputs):
    global LAST_RESULT
    in_maps = _prep_inputs(inputs)
    if 'nc' not in _NC_CACHE:
        _NC_CACHE['nc'] = build_nc()
    nc = _NC_CACHE['nc']
    res = run_bass_kernel_spmd(nc, in_maps, core_ids=list(range(NCORES)),
                               trace=TRACE)
    LAST_RESULT = res
    y = np.concatenate([res.results[c]['y'] for c in range(NCORES)], axis=0)
    return y.reshape(B, N, 1).astype(np.float32)


if __name__ == "__main__":
    data = np.load('/root/problem/work/inputs.npz')
    inp = {k: data[k] for k in data.files}
    y = kernel(**inp)
    exp = np.load('/root/problem/work/expected.npy')
    err = np.abs(y - exp).max()
    print("max abs err:", err, "rel:", err / np.abs(exp).max())
